# revision 9
# baseline (speedup 1.0000x reference)
"""AttentionPool (segment softmax + weighted scatter-add) on 8 trn2 NeuronCores.

Strategy
--------
Segment-ALIGNED sharding: batch ids are sorted, and B = 1024 = 8 * 128, so
core c owns segments [128c, 128(c+1)) exactly.  Host computes the row range
of each core with searchsorted, so no cross-core collective is needed at all
-- each core produces a disjoint (128, 128) slice of the output.

This runtime has a large (~120-220 ns) per-instruction issue overhead, so
the design minimizes instruction count: per-tile work is only one DVE
logits op + one PE matmul; the one-hot build is batched into 2 WIDE DVE
ops per group using stride-0 broadcast access patterns.

Per core (T row-tiles of 128 rows, grouped into groups of G tiles):
  1. DMA x in big chunks as BF16 (host pre-packs x into the SBUF layout
     (128, T*130): per tile 128 x-columns + a ones column + 1 pad col).
     bf16 halves the HBM traffic vs f32 (the memory roofline here).
  2. logits: DVE scalar_tensor_tensor  scr=(x*1)*Wrep with accum_out
     -> l[p] = sum_d x[p,d]*W[d].  (native ISA; tensor_tensor_reduce is a
     custom-DVE op that hangs under this axon runtime.)
  3. e = exp(l + b): one ACT instruction per group (bf16 out).
  4. WIDE unscaled one-hot for the whole group (1 DVE op):
       oh0[p, t, s] = (slot[p, t] == iota[s])   via scalar_tensor_tensor
     with slot broadcast along s (stride-0) and a host-packed iota_rep.
  5. WIDE e-scaling (1 DVE op): oh[p, t, s] = oh0[p, t, s] * e[p, t]
     with e broadcast along s (stride-0).
  6. PE: psum (S, 130) += oh_t^T @ [x | 1]  accumulated over the group's
     G tiles in bf16 (1 cyc/row; fp32 is 4 cyc/row, float32r hangs here).
  7. per group: ACT-copy psum -> SBUF staging (bf16), then immediately
     scatter-add into the (128,130) fps psum with a small bf16 one-hot
     matmul (overlapped with later groups).
  8. final: v/(s + 1e-16); DMA out.

The kernel() entry point takes FULL inputs and returns the FULL (1024, 128)
output; it validates the device result against a float64 numpy reference
on the host and falls back to the exact-f32 numeric config if the bf16
gate fails.
"""

import os
import sys

import numpy as np

for _p in ("/root/.axon_site", "/root/.axon_site/_ro/trn_rl_repo", "/root/.axon_site/_ro/pypackages"):
    if os.path.isdir(_p) and _p not in sys.path:
        sys.path.append(_p)

from contextlib import ExitStack

import ml_dtypes

import concourse.bacc as bacc
import concourse.tile as tile
from concourse import mybir
from concourse.bass_utils import run_bass_kernel_spmd

N_CORES = 8
D = 128
TPT = 130  # columns per tile in the packed x layout: 128 x + 1 ones + 1 pad

Alu = mybir.AluOpType
Act = mybir.ActivationFunctionType
F32 = mybir.dt.float32
BF16 = mybir.dt.bfloat16
NP_BF16 = ml_dtypes.bfloat16

_program_cache: dict = {}


def _b3(ap, S):
    """(P, T) AP -> (P, T, S) with stride-0 broadcast along s."""
    return ap.unsqueeze(2).broadcast_to([ap.shape[0], ap.shape[1], S])


def build_program(T, G, S, n_groups, mm_dtype="bf16", n_dma_per_group=2,
                  reps=1, bufs_x=3):
    """Build the per-core bass program (same program for all 8 cores)."""
    key = (T, G, S, n_groups, mm_dtype, n_dma_per_group, reps, bufs_x)
    if key in _program_cache:
        return _program_cache[key]

    assert n_groups == (T + G - 1) // G
    nc = bacc.Bacc("TRN2", target_bir_lowering=False)

    bf16 = mm_dtype == "bf16"
    XDT = BF16 if bf16 else F32

    x_in = nc.declare_dram_parameter("xs", [128, T * TPT], XDT, isOutput=False)
    slots_in = nc.declare_dram_parameter("slots", [128, T], XDT, isOutput=False)
    fslots_in = nc.declare_dram_parameter("fslots", [S, n_groups], F32, isOutput=False)
    wrep_in = nc.declare_dram_parameter("wrep", [128, TPT], XDT, isOutput=False)
    brep_in = nc.declare_dram_parameter("brep", [128, 1], F32, isOutput=False)
    iota_rep_in = nc.declare_dram_parameter("iota_rep", [128, S * G], XDT, isOutput=False)
    iota_m_in = nc.declare_dram_parameter("iota_m", [S, 128], XDT, isOutput=False)
    y_out = nc.declare_dram_parameter("out", [128, 128], F32, isOutput=True)

    with tile.TileContext(nc) as tc:
        with ExitStack() as ctx:
            cpool = ctx.enter_context(tc.tile_pool(name="consts", bufs=1))
            xpool = ctx.enter_context(tc.tile_pool(name="x", bufs=bufs_x))
            spool = ctx.enter_context(tc.tile_pool(name="scr", bufs=2))
            lpool = ctx.enter_context(tc.tile_pool(name="l", bufs=2))
            epool = ctx.enter_context(tc.tile_pool(name="e", bufs=2))
            oh0pool = ctx.enter_context(tc.tile_pool(name="oh0", bufs=2))
            ohpool = ctx.enter_context(tc.tile_pool(name="oh", bufs=2))
            pspool = ctx.enter_context(tc.tile_pool(name="ps", bufs=4, space="PSUM"))
            stpool = ctx.enter_context(tc.tile_pool(name="stage", bufs=2))
            fohpool = ctx.enter_context(tc.tile_pool(name="foh", bufs=2))
            fpool = ctx.enter_context(tc.tile_pool(name="fin", bufs=1, space="PSUM"))
            opool = ctx.enter_context(tc.tile_pool(name="outp", bufs=1))

            wrep = cpool.tile([128, TPT], XDT)
            nc.sync.dma_start(wrep[:], wrep_in[:])
            brep = cpool.tile([128, 1], F32)
            nc.sync.dma_start(brep[:], brep_in[:])
            iota_rep = cpool.tile([128, S * G], XDT)
            nc.sync.dma_start(iota_rep[:], iota_rep_in[:])
            iota_m = cpool.tile([S, 128], XDT)
            nc.sync.dma_start(iota_m[:], iota_m_in[:])
            slots = cpool.tile([128, T], XDT)
            nc.sync.dma_start(slots[:], slots_in[:])
            fslots = cpool.tile([S, n_groups], F32)
            nc.sync.dma_start(fslots[:], fslots_in[:])

            # all groups' final-scatter one-hots in ONE wide op, hoisted out
            # of the reps loop entirely (depends only on consts)
            fohs = cpool.tile([S, n_groups * 128], XDT)
            nc.vector.scalar_tensor_tensor(
                fohs[:].rearrange("p (g m) -> p g m", m=128),
                iota_m[:].unsqueeze(1).broadcast_to([S, n_groups, 128]),
                1.0,
                _b3(fslots[:], 128),
                Alu.mult,
                Alu.is_equal,
            )

            def emit_body():
                fps = fpool.tile([128, TPT], F32, tag="fps")
                for g in range(n_groups):
                    Gg = min(G, T - g * G)
                    xc = xpool.tile([128, G * TPT], XDT, tag="xc")
                    # load this group's packed x (Gg*TPT cols) in pieces
                    cols = Gg * TPT
                    step = (cols + n_dma_per_group - 1) // n_dma_per_group
                    for k in range(0, cols, step):
                        w = min(step, cols - k)
                        nc.sync.dma_start(
                            xc[:, k : k + w],
                            x_in[:, g * G * TPT + k : g * G * TPT + k + w],
                        )
                    # logits, 2-pass wide: (1) scr = x * Wrep (one DVE op,
                    # all operands packed bf16 -> DVE fast mode; W pattern
                    # has zeros at the ones/pad columns), (2) 3D
                    # tensor_reduce over the innermost 130 -> per-tile sums.
                    scr_w = spool.tile([128, G * TPT], XDT, tag="scrw")
                    nc.vector.scalar_tensor_tensor(
                        scr_w[:, 0 : Gg * TPT].rearrange("p (t c) -> p t c", c=TPT),
                        xc[:, 0 : Gg * TPT].rearrange("p (t c) -> p t c", c=TPT),
                        1.0,
                        wrep[:].unsqueeze(1).broadcast_to([128, Gg, TPT]),
                        Alu.mult,
                        Alu.mult,
                    )
                    l_t = lpool.tile([128, Gg], F32, tag="l")
                    nc.vector.tensor_reduce(
                        l_t[:],
                        scr_w[:, 0 : Gg * TPT].rearrange("p (t c) -> p t c", c=TPT),
                        mybir.AxisListType.X,
                        Alu.add,
                    )
                    e_t = epool.tile([128, Gg], XDT, tag="e")
                    nc.scalar.activation(e_t[:], l_t[:], Act.Exp, bias=brep[:], scale=1.0)
                    # WIDE one-hot build, s-major so every operand is packed
                    # (the broadcast axes are the middle dim, stride-0):
                    #   oh[p, s, t] = (iota_rep[p, s*G+t] == slot[p, t]) * e[p, t]
                    oh0 = oh0pool.tile([128, S * G], XDT, tag="oh0")
                    nc.vector.scalar_tensor_tensor(
                        oh0[:].rearrange("p (s t) -> p s t", t=G)[:, :, 0:Gg],
                        iota_rep[:].rearrange("p (s t) -> p s t", t=G)[:, :, 0:Gg],
                        1.0,
                        slots[:, g * G : g * G + Gg].unsqueeze(1).broadcast_to([128, S, Gg]),
                        Alu.mult,
                        Alu.is_equal,
                    )
                    oh = ohpool.tile([128, S * G], XDT, tag="oh")
                    nc.vector.scalar_tensor_tensor(
                        oh[:].rearrange("p (s t) -> p s t", t=G)[:, :, 0:Gg],
                        oh0[:].rearrange("p (s t) -> p s t", t=G)[:, :, 0:Gg],
                        1.0,
                        e_t[:].unsqueeze(1).broadcast_to([128, S, Gg]),
                        Alu.mult,
                        Alu.mult,
                    )
                    # per-tile scatter matmuls; lhsT is an s-major strided
                    # slice: column s of tile t lives at oh[:, s*G + t]
                    ps = pspool.tile([S, TPT], F32, tag="ps")
                    for t in range(Gg):
                        nc.tensor.matmul(
                            ps[:],
                            lhsT=oh[:, t : t + (S - 1) * G + 1 : G],
                            rhs=xc[:, t * TPT : t * TPT + TPT],
                            start=(t == 0),
                            stop=(t == Gg - 1),
                        )
                    staging = stpool.tile([S, TPT], XDT, tag="stage")
                    nc.scalar.copy(staging[:], ps[:])
                    # scatter-add this group's partial into the (128,*) psum
                    nc.tensor.matmul(
                        fps[:],
                        lhsT=fohs[:, g * 128 : (g + 1) * 128],
                        rhs=staging[:],
                        start=(g == 0),
                        stop=(g == n_groups - 1),
                    )
                s_plus = opool.tile([128, 1], F32, tag="sp")
                nc.vector.tensor_scalar_add(s_plus[:], fps[:, 128:129], 1e-16)
                recip = opool.tile([128, 1], F32, tag="rc")
                nc.vector.reciprocal(recip[:], s_plus[:])
                out_sb = opool.tile([128, 128], F32, tag="ot")
                nc.vector.tensor_scalar(
                    out_sb[:], fps[:, 0:128], recip[:], None, Alu.mult
                )
                nc.sync.dma_start(y_out[:], out_sb[:])

            if reps == 1:
                emit_body()
            else:
                with tc.For_i(0, reps, 1):
                    emit_body()

    nc.finalize()
    _program_cache[key] = nc
    return nc


def prepare_shards(x, batch, W, b, B, S=32, G=64, mm_dtype="bf16"):
    """Host-side packing. Returns (in_maps, meta)."""
    x = np.asarray(x, dtype=np.float32)
    batch = np.asarray(batch).astype(np.int64)
    W = np.asarray(W, dtype=np.float32)
    b = np.asarray(b, dtype=np.float32)
    np_xdt = NP_BF16 if mm_dtype == "bf16" else np.float32
    N = x.shape[0]
    segs_per_core = B // N_CORES
    bounds = np.searchsorted(batch, np.arange(0, B + 1, segs_per_core))
    T = int(max(-(-(int(bounds[c + 1] - bounds[c])) // 128) for c in range(N_CORES)))

    # pick G such that every group's segment span fits in S slots
    loc_all = batch - (batch // segs_per_core) * segs_per_core
    while G > 1:
        ok = True
        for c in range(N_CORES):
            r0, r1 = int(bounds[c]), int(bounds[c + 1])
            n = r1 - r0
            if n == 0:
                continue
            loc = loc_all[r0:r1]
            g_idx = np.arange(n) // (G * 128)
            gstart = np.minimum(np.arange(g_idx[-1] + 1) * G * 128, n - 1)
            gb = loc[gstart]
            span = loc - gb[g_idx]
            if span.min() < 0 or span.max() >= S:
                ok = False
                break
        if ok:
            break
        G //= 2
    n_groups = (T + G - 1) // G

    wpat = np.zeros(TPT, np.float32)
    wpat[:128] = W[:, 0]
    wrep = np.tile(wpat[None, :], (128, 1)).astype(np_xdt)
    brep = np.full((128, 1), float(b[0]), np.float32)
    # s-major iota: value s at position s*G + t
    iota_rep = np.tile(
        np.repeat(np.arange(S, dtype=np.float32), G)[None, :], (128, 1)
    ).astype(np_xdt)
    iota_m = np.tile(np.arange(128, dtype=np.float32)[None, :], (S, 1)).astype(np_xdt)

    in_maps = []
    for c in range(N_CORES):
        r0, r1 = int(bounds[c]), int(bounds[c + 1])
        n = r1 - r0
        xp = np.zeros((T * 128, TPT), np_xdt)
        xp[:n, :128] = x[r0:r1].astype(np_xdt)
        xp[:n, 128] = 1.0
        x_shard = np.ascontiguousarray(
            xp.reshape(T, 128, TPT).transpose(1, 0, 2).reshape(128, T * TPT)
        )

        slots_full = np.full(T * 128, -1.0, np.float32)
        fslots = np.full((S, n_groups), -1.0, np.float32)
        if n > 0:
            loc = loc_all[r0:r1]
            g_idx = np.arange(n) // (G * 128)
            ng_real = int(g_idx[-1]) + 1
            gstart = np.minimum(np.arange(ng_real) * G * 128, n - 1)
            gb = loc[gstart]
            slot = loc - gb[g_idx]
            assert slot.min() >= 0 and slot.max() < S
            slots_full[:n] = slot.astype(np.float32)  # ints <= S fit bf16 exactly
            for g in range(ng_real):
                segs = gb[g] + np.arange(S)
                valid = segs < segs_per_core
                fslots[valid, g] = segs[valid].astype(np.float32)
        slots_T = np.ascontiguousarray(slots_full.reshape(T, 128).T).astype(np_xdt)

        in_maps.append(
            {
                "xs": x_shard,
                "slots": slots_T,
                "fslots": fslots,
                "wrep": wrep,
                "brep": brep,
                "iota_rep": iota_rep,
                "iota_m": iota_m,
            }
        )
    meta = dict(T=T, G=G, S=S, n_groups=n_groups, segs_per_core=segs_per_core)
    return in_maps, meta


def _ref_numpy(x, batch, W, b, B):
    """Float64 host reference (same math as the jax oracle) used only as a
    validation gate for the on-device numeric mode."""
    x = np.asarray(x, np.float64)
    batch = np.asarray(batch).astype(np.int64)
    logits = x @ np.asarray(W, np.float64)[:, 0] + float(np.asarray(b)[0])
    starts = np.searchsorted(batch, np.arange(B))
    counts = np.bincount(batch, minlength=B)
    # segment max (batch sorted -> reduceat over contiguous runs)
    valid = counts > 0
    seg_max = np.zeros(B)
    seg_max[valid] = np.maximum.reduceat(logits, starts[valid])[: valid.sum()]
    # reduceat quirk: rows with equal consecutive starts handled via `valid`
    e = np.exp(logits - seg_max[batch])
    seg_sum = np.zeros(B)
    seg_sum[valid] = np.add.reduceat(e, starts[valid])[: valid.sum()]
    w = e / (seg_sum[batch] + 1e-16)
    wx = w[:, None] * x
    out = np.zeros((B, x.shape[1]))
    out[valid] = np.add.reduceat(wx, starts[valid], axis=0)[: valid.sum()]
    return out


def kernel(x, batch, W, b, num_graphs):
    B = int(num_graphs)
    ref = _ref_numpy(x, batch, W, b, B)
    scale = max(1e-30, float(np.abs(ref).max()))
    best = None
    for mm in ("bf16", "f32"):
        in_maps, meta = prepare_shards(x, batch, W, b, B, mm_dtype=mm)
        nc = build_program(meta["T"], meta["G"], meta["S"], meta["n_groups"],
                           mm_dtype=mm)
        res = run_bass_kernel_spmd(nc, in_maps, core_ids=list(range(N_CORES)))
        out = np.concatenate(
            [res.results[c]["out"] for c in range(N_CORES)], axis=0
        ).astype(np.float32)
        rel = float(np.abs(np.asarray(out, np.float64) - ref).max() / scale)
        if best is None or rel < best[1]:
            best = (out, rel)
        if rel < 1.1e-2:
            return out
    return best[0]


# revision 30
# speedup vs baseline: 1.1937x; 1.1937x over previous
"""AttentionPool (segment softmax + weighted scatter-add) on 8 trn2 NeuronCores.

Strategy
--------
Segment-ALIGNED sharding: batch ids are sorted, and B = 1024 = 8 * 128, so
core c owns segments [128c, 128(c+1)) exactly.  Host computes the row range
of each core with searchsorted, so no cross-core collective is needed at all
-- each core produces a disjoint (128, 128) slice of the output.

This runtime has a large (~120-220 ns) per-instruction issue overhead, so
the design minimizes instruction count: per-tile work is only one DVE
logits op + one PE matmul; the one-hot build is batched into 2 WIDE DVE
ops per group using stride-0 broadcast access patterns.

Per core (T row-tiles of 128 rows, grouped into groups of G tiles):
  1. DMA x in big chunks as BF16 (host pre-packs x into the SBUF layout
     (128, T*130): per tile 128 x-columns + a ones column + 1 pad col).
     bf16 halves the HBM traffic vs f32 (the memory roofline here).
  2. logits: DVE scalar_tensor_tensor  scr=(x*1)*Wrep with accum_out
     -> l[p] = sum_d x[p,d]*W[d].  (native ISA; tensor_tensor_reduce is a
     custom-DVE op that hangs under this axon runtime.)
  3. e = exp(l + b): one ACT instruction per group (bf16 out).
  4. WIDE unscaled one-hot for the whole group (1 DVE op):
       oh0[p, t, s] = (slot[p, t] == iota[s])   via scalar_tensor_tensor
     with slot broadcast along s (stride-0) and a host-packed iota_rep.
  5. WIDE e-scaling (1 DVE op): oh[p, t, s] = oh0[p, t, s] * e[p, t]
     with e broadcast along s (stride-0).
  6. PE: psum (S, 130) += oh_t^T @ [x | 1]  accumulated over the group's
     G tiles in bf16 (1 cyc/row; fp32 is 4 cyc/row, float32r hangs here).
  7. per group: ACT-copy psum -> SBUF staging (bf16), then immediately
     scatter-add into the (128,130) fps psum with a small bf16 one-hot
     matmul (overlapped with later groups).
  8. final: v/(s + 1e-16); DMA out.

The kernel() entry point takes FULL inputs and returns the FULL (1024, 128)
output; it validates the device result against a float64 numpy reference
on the host and falls back to the exact-f32 numeric config if the bf16
gate fails.
"""

import os
import sys

import numpy as np

for _p in ("/root/.axon_site", "/root/.axon_site/_ro/trn_rl_repo", "/root/.axon_site/_ro/pypackages"):
    if os.path.isdir(_p) and _p not in sys.path:
        sys.path.append(_p)

from contextlib import ExitStack

import ml_dtypes

import concourse.bacc as bacc
import concourse.tile as tile
from concourse import mybir
from concourse.bass_utils import run_bass_kernel_spmd

N_CORES = 8
D = 128
TPT = 130  # columns per tile in the packed x layout: 128 x + 1 ones + 1 pad

Alu = mybir.AluOpType
Act = mybir.ActivationFunctionType
F32 = mybir.dt.float32
BF16 = mybir.dt.bfloat16
NP_BF16 = ml_dtypes.bfloat16

_program_cache: dict = {}


def _b3(ap, S):
    """(P, T) AP -> (P, T, S) with stride-0 broadcast along s."""
    return ap.unsqueeze(2).broadcast_to([ap.shape[0], ap.shape[1], S])


def build_program(T, G, S, n_groups, mm_dtype="bf16", n_dma_per_group=2,
                  reps=1, bufs_x=6, reduce_mode="stt", oh_major="s",
                  pipelined=False, ladder=7, oh_engine="vector"):
    """Build the per-core bass program (same program for all 8 cores).

    reduce_mode: "tree" = log2 ladder of packed bf16 adds (DVE fast mode);
                 "tr"   = single 3D tensor_reduce (runs at 1x rate).
    oh_major:    "s" = s-major one-hot (strided matmul lhsT);
                 "t" = t-major (contiguous lhsT, stride-0 inner broadcasts).
    pipelined:   emit group g's e-scale/matmuls during group g+1 so the
                 in-order DVE/PE streams never stall on the ACT exp/copy.
    """
    key = (T, G, S, n_groups, mm_dtype, n_dma_per_group, reps, bufs_x,
           reduce_mode, oh_major, pipelined, ladder, oh_engine)
    if key in _program_cache:
        return _program_cache[key]

    assert n_groups == (T + G - 1) // G
    nc = bacc.Bacc("TRN2", target_bir_lowering=False)

    bf16 = mm_dtype == "bf16"
    XDT = BF16 if bf16 else F32

    x_in = nc.declare_dram_parameter("xs", [128, T * TPT], XDT, isOutput=False)
    slots_in = nc.declare_dram_parameter("slots", [128, T], XDT, isOutput=False)
    fslots_in = nc.declare_dram_parameter("fslots", [S, n_groups], F32, isOutput=False)
    wrep_in = nc.declare_dram_parameter("wrep", [128, TPT], XDT, isOutput=False)
    brep_in = nc.declare_dram_parameter("brep", [128, 1], F32, isOutput=False)
    iota_rep_in = nc.declare_dram_parameter("iota_rep", [128, S * G], XDT, isOutput=False)
    iota_rep_t_in = nc.declare_dram_parameter("iota_rep_t", [128, G * S], XDT, isOutput=False)
    iota_m_in = nc.declare_dram_parameter("iota_m", [S, 128], XDT, isOutput=False)
    y_out = nc.declare_dram_parameter("out", [128, 128], F32, isOutput=True)

    with tile.TileContext(nc) as tc:
        with ExitStack() as ctx:
            cpool = ctx.enter_context(tc.tile_pool(name="consts", bufs=1))
            xpool = ctx.enter_context(tc.tile_pool(name="x", bufs=bufs_x))
            spool = ctx.enter_context(tc.tile_pool(name="scr", bufs=4))
            tpool = ctx.enter_context(tc.tile_pool(name="tree", bufs=1))
            lpool = ctx.enter_context(tc.tile_pool(name="l", bufs=2))
            epool = ctx.enter_context(tc.tile_pool(name="e", bufs=2))
            oh0pool = ctx.enter_context(tc.tile_pool(name="oh0", bufs=2))
            ohpool = ctx.enter_context(tc.tile_pool(name="oh", bufs=2))
            pspool = ctx.enter_context(tc.tile_pool(name="ps", bufs=4, space="PSUM"))
            stpool = ctx.enter_context(tc.tile_pool(name="stage", bufs=2))
            fpool = ctx.enter_context(tc.tile_pool(name="fin", bufs=1, space="PSUM"))
            opool = ctx.enter_context(tc.tile_pool(name="outp", bufs=1))

            wrep = cpool.tile([128, TPT], XDT)
            nc.sync.dma_start(wrep[:], wrep_in[:])
            brep = cpool.tile([128, 1], F32)
            nc.sync.dma_start(brep[:], brep_in[:])
            iota_rep = cpool.tile([128, S * G], XDT)
            nc.sync.dma_start(iota_rep[:], iota_rep_in[:])
            iota_rep_t = cpool.tile([128, G * S], XDT)
            nc.sync.dma_start(iota_rep_t[:], iota_rep_t_in[:])
            iota_m = cpool.tile([S, 128], XDT)
            nc.sync.dma_start(iota_m[:], iota_m_in[:])
            slots = cpool.tile([128, T], XDT)
            nc.sync.dma_start(slots[:], slots_in[:])
            fslots = cpool.tile([S, n_groups], F32)
            nc.sync.dma_start(fslots[:], fslots_in[:])

            # all groups' final-scatter one-hots in ONE wide op, hoisted out
            # of the reps loop entirely (depends only on consts)
            fohs = cpool.tile([S, n_groups * 128], XDT)
            nc.vector.scalar_tensor_tensor(
                fohs[:].rearrange("p (g m) -> p g m", m=128),
                iota_m[:].unsqueeze(1).broadcast_to([S, n_groups, 128]),
                1.0,
                _b3(fslots[:], 128),
                Alu.mult,
                Alu.is_equal,
            )

            def _front_tail(g, Gg, xc, l_t):
                if ladder < 3:
                    return dict(g=g, Gg=Gg, xc=xc, oh0=None, e_t=None)
                e_t = epool.tile([128, Gg], XDT, tag="e")
                nc.scalar.activation(e_t[:], l_t[:], Act.Exp, bias=brep[:], scale=1.0)
                if ladder < 4:
                    return dict(g=g, Gg=Gg, xc=xc, oh0=None, e_t=e_t)
                # unscaled one-hot (depends only on consts)
                oh0 = oh0pool.tile([128, S * G], XDT, tag="oh0")
                ohe = nc.gpsimd if oh_engine == "gpsimd" else nc.vector
                if oh_major == "s":
                    # oh0[p, s*G + t] = (iota_rep[p, s*G+t] == slot[p, t])
                    ohe.scalar_tensor_tensor(
                        oh0[:].rearrange("p (s t) -> p s t", t=G)[:, :, 0:Gg],
                        iota_rep[:].rearrange("p (s t) -> p s t", t=G)[:, :, 0:Gg],
                        1.0,
                        slots[:, g * G : g * G + Gg].unsqueeze(1).broadcast_to([128, S, Gg]),
                        Alu.mult,
                        Alu.is_equal,
                    )
                else:
                    # oh0[p, t*S + s] = (iota_rep_t[p, t*S+s] == slot[p, t])
                    ohe.scalar_tensor_tensor(
                        oh0[:, 0 : Gg * S].rearrange("p (t s) -> p t s", s=S),
                        iota_rep_t[:, 0 : Gg * S].rearrange("p (t s) -> p t s", s=S),
                        1.0,
                        _b3(slots[:, g * G : g * G + Gg], S),
                        Alu.mult,
                        Alu.is_equal,
                    )
                return dict(g=g, Gg=Gg, xc=xc, oh0=oh0, e_t=e_t)

            def emit_front_half(g):
                """DMA + logits (mult, reduce) + exp + unscaled one-hot for
                group g.  Everything here depends only on xc/consts, so the
                in-order DVE stream never waits on another engine."""
                Gg = min(G, T - g * G)
                xc = xpool.tile([128, G * TPT], XDT, tag="xc")
                cols = Gg * TPT
                step = (cols + n_dma_per_group - 1) // n_dma_per_group
                for k in range(0, cols, step):
                    w = min(step, cols - k)
                    nc.sync.dma_start(
                        xc[:, k : k + w],
                        x_in[:, g * G * TPT + k : g * G * TPT + k + w],
                    )
                if ladder < 2:
                    return dict(g=g, Gg=Gg, xc=xc, oh0=None, e_t=None)
                if reduce_mode == "stt":
                    # fused multiply+accumulate per tile: ONE pass over x
                    # (the 2-pass wide variants touch the data twice)
                    l_t = lpool.tile([128, Gg], F32, tag="l")
                    for t in range(Gg):
                        scr = spool.tile([128, 128], XDT, tag="scr")
                        nc.vector.scalar_tensor_tensor(
                            scr[:],
                            xc[:, t * TPT : t * TPT + 128],
                            1.0,
                            wrep[:, 0:128],
                            Alu.mult,
                            Alu.mult,
                            accum_out=l_t[:, t : t + 1],
                        )
                    return _front_tail(g, Gg, xc, l_t)
                # logits pass 1: scr = x * Wrep (all operands packed bf16;
                # W pattern has zeros at the ones/pad columns)
                scr_w = spool.tile([128, G * TPT], XDT, tag="scrw")
                scr3 = scr_w[:, 0 : Gg * TPT].rearrange("p (t c) -> p t c", c=TPT)
                nc.vector.scalar_tensor_tensor(
                    scr3,
                    xc[:, 0 : Gg * TPT].rearrange("p (t c) -> p t c", c=TPT),
                    1.0,
                    wrep[:].unsqueeze(1).broadcast_to([128, Gg, TPT]),
                    Alu.mult,
                    Alu.mult,
                )
                # logits pass 2: per-tile row sums of the first 128 columns
                l_t = lpool.tile([128, Gg], F32, tag="l")
                if reduce_mode == "tr":
                    nc.vector.tensor_reduce(
                        l_t[:], scr3, mybir.AxisListType.X, Alu.add
                    )
                else:
                    # log2 ladder of packed bf16 adds (DVE fast mode);
                    # level widths 64,32,16,8,4,2 then a final fp32 add.
                    tree = tpool.tile([128, G * 126], XDT, tag="tree")
                    off = 0
                    src3 = scr3
                    w = 64
                    while w >= 2:
                        dst = tree[:, off * G : off * G + Gg * w]
                        dst3 = dst.rearrange("p (t c) -> p t c", c=w)
                        nc.vector.scalar_tensor_tensor(
                            dst3,
                            src3[:, :, 0:w],
                            1.0,
                            src3[:, :, w : 2 * w],
                            Alu.mult,
                            Alu.add,
                        )
                        src3 = dst3
                        off += w
                        w //= 2
                    nc.vector.scalar_tensor_tensor(
                        l_t[:].unsqueeze(2),
                        src3[:, :, 0:1],
                        1.0,
                        src3[:, :, 1:2],
                        Alu.mult,
                        Alu.add,
                    )
                return _front_tail(g, Gg, xc, l_t)

            def emit_back_half(st):
                """e-scaling + scatter matmuls + fold-in for group st['g'].
                Emitted one group late so exp/copy results are ready."""
                g, Gg, xc, oh0, e_t = st["g"], st["Gg"], st["xc"], st["oh0"], st["e_t"]
                if ladder < 5:
                    return None
                oh = ohpool.tile([128, S * G], XDT, tag="oh")
                ohe = nc.gpsimd if oh_engine == "gpsimd" else nc.vector
                if oh_major == "s":
                    ohe.scalar_tensor_tensor(
                        oh[:].rearrange("p (s t) -> p s t", t=G)[:, :, 0:Gg],
                        oh0[:].rearrange("p (s t) -> p s t", t=G)[:, :, 0:Gg],
                        1.0,
                        e_t[:].unsqueeze(1).broadcast_to([128, S, Gg]),
                        Alu.mult,
                        Alu.mult,
                    )
                else:
                    ohe.scalar_tensor_tensor(
                        oh[:, 0 : Gg * S].rearrange("p (t s) -> p t s", s=S),
                        oh0[:, 0 : Gg * S].rearrange("p (t s) -> p t s", s=S),
                        1.0,
                        _b3(e_t[:], S),
                        Alu.mult,
                        Alu.mult,
                    )
                if ladder < 6:
                    return None
                ps = pspool.tile([S, TPT], F32, tag="ps")
                for t in range(Gg):
                    if oh_major == "s":
                        lhsT = oh[:, t : t + (S - 1) * G + 1 : G]
                    else:
                        lhsT = oh[:, t * S : (t + 1) * S]
                    nc.tensor.matmul(
                        ps[:],
                        lhsT=lhsT,
                        rhs=xc[:, t * TPT : t * TPT + TPT],
                        start=(t == 0),
                        stop=(t == Gg - 1),
                    )
                staging = stpool.tile([S, TPT], XDT, tag="stage")
                nc.scalar.copy(staging[:], ps[:])
                return dict(g=g, staging=staging)

            def emit_fold(fps, fin_st):
                if fin_st is None or ladder < 7:
                    return
                g, staging = fin_st["g"], fin_st["staging"]
                nc.tensor.matmul(
                    fps[:],
                    lhsT=fohs[:, g * 128 : (g + 1) * 128],
                    rhs=staging[:],
                    start=(g == 0),
                    stop=(g == n_groups - 1),
                )

            def emit_body():
                fps = fpool.tile([128, TPT], F32, tag="fps")
                if pipelined:
                    pending = None
                    pending_fold = None
                    for g in range(n_groups):
                        st = emit_front_half(g)
                        if pending is not None:
                            fin_st = emit_back_half(pending)
                            if pending_fold is not None:
                                emit_fold(fps, pending_fold)
                            pending_fold = fin_st
                        pending = st
                    fin_st = emit_back_half(pending)
                    if pending_fold is not None:
                        emit_fold(fps, pending_fold)
                    emit_fold(fps, fin_st)
                else:
                    for g in range(n_groups):
                        st = emit_front_half(g)
                        fin_st = emit_back_half(st)
                        emit_fold(fps, fin_st)
                if ladder < 7:
                    out_sb = opool.tile([128, 128], F32, tag="ot")
                    nc.vector.memset(out_sb[:], 0.0)
                    nc.sync.dma_start(y_out[:], out_sb[:])
                    return
                s_plus = opool.tile([128, 1], F32, tag="sp")
                nc.vector.tensor_scalar_add(s_plus[:], fps[:, 128:129], 1e-16)
                recip = opool.tile([128, 1], F32, tag="rc")
                nc.vector.reciprocal(recip[:], s_plus[:])
                out_sb = opool.tile([128, 128], F32, tag="ot")
                nc.vector.tensor_scalar(
                    out_sb[:], fps[:, 0:128], recip[:], None, Alu.mult
                )
                nc.sync.dma_start(y_out[:], out_sb[:])

            if reps == 1:
                emit_body()
            else:
                with tc.For_i(0, reps, 1):
                    emit_body()

    nc.finalize()
    _program_cache[key] = nc
    return nc


def prepare_shards(x, batch, W, b, B, S=16, G=64, mm_dtype="bf16"):
    """Host-side packing. Returns (in_maps, meta)."""
    x = np.asarray(x, dtype=np.float32)
    batch = np.asarray(batch).astype(np.int64)
    W = np.asarray(W, dtype=np.float32)
    b = np.asarray(b, dtype=np.float32)
    np_xdt = NP_BF16 if mm_dtype == "bf16" else np.float32
    N = x.shape[0]
    segs_per_core = B // N_CORES
    bounds = np.searchsorted(batch, np.arange(0, B + 1, segs_per_core))
    T = int(max(-(-(int(bounds[c + 1] - bounds[c])) // 128) for c in range(N_CORES)))

    # pick G such that every group's segment span fits in S slots
    loc_all = batch - (batch // segs_per_core) * segs_per_core
    while G > 1:
        ok = True
        for c in range(N_CORES):
            r0, r1 = int(bounds[c]), int(bounds[c + 1])
            n = r1 - r0
            if n == 0:
                continue
            loc = loc_all[r0:r1]
            g_idx = np.arange(n) // (G * 128)
            gstart = np.minimum(np.arange(g_idx[-1] + 1) * G * 128, n - 1)
            gb = loc[gstart]
            span = loc - gb[g_idx]
            if span.min() < 0 or span.max() >= S:
                ok = False
                break
        if ok:
            break
        G //= 2
    n_groups = (T + G - 1) // G

    wpat = np.zeros(TPT, np.float32)
    wpat[:128] = W[:, 0]
    wrep = np.tile(wpat[None, :], (128, 1)).astype(np_xdt)
    brep = np.full((128, 1), float(b[0]), np.float32)
    # s-major iota: value s at position s*G + t
    iota_rep = np.tile(
        np.repeat(np.arange(S, dtype=np.float32), G)[None, :], (128, 1)
    ).astype(np_xdt)
    # t-major iota: value s at position t*S + s
    iota_rep_t = np.tile(
        np.arange(S, dtype=np.float32)[None, :], (128, G)
    ).astype(np_xdt)
    iota_m = np.tile(np.arange(128, dtype=np.float32)[None, :], (S, 1)).astype(np_xdt)

    in_maps = []
    for c in range(N_CORES):
        r0, r1 = int(bounds[c]), int(bounds[c + 1])
        n = r1 - r0
        xp = np.zeros((T * 128, TPT), np_xdt)
        xp[:n, :128] = x[r0:r1].astype(np_xdt)
        xp[:n, 128] = 1.0
        x_shard = np.ascontiguousarray(
            xp.reshape(T, 128, TPT).transpose(1, 0, 2).reshape(128, T * TPT)
        )

        slots_full = np.full(T * 128, -1.0, np.float32)
        fslots = np.full((S, n_groups), -1.0, np.float32)
        if n > 0:
            loc = loc_all[r0:r1]
            g_idx = np.arange(n) // (G * 128)
            ng_real = int(g_idx[-1]) + 1
            gstart = np.minimum(np.arange(ng_real) * G * 128, n - 1)
            gb = loc[gstart]
            slot = loc - gb[g_idx]
            assert slot.min() >= 0 and slot.max() < S
            slots_full[:n] = slot.astype(np.float32)  # ints <= S fit bf16 exactly
            for g in range(ng_real):
                segs = gb[g] + np.arange(S)
                valid = segs < segs_per_core
                fslots[valid, g] = segs[valid].astype(np.float32)
        slots_T = np.ascontiguousarray(slots_full.reshape(T, 128).T).astype(np_xdt)

        in_maps.append(
            {
                "xs": x_shard,
                "slots": slots_T,
                "fslots": fslots,
                "wrep": wrep,
                "brep": brep,
                "iota_rep": iota_rep,
                "iota_rep_t": iota_rep_t,
                "iota_m": iota_m,
            }
        )
    meta = dict(T=T, G=G, S=S, n_groups=n_groups, segs_per_core=segs_per_core)
    return in_maps, meta


def _ref_numpy(x, batch, W, b, B):
    """Float64 host reference (same math as the jax oracle) used only as a
    validation gate for the on-device numeric mode."""
    x = np.asarray(x, np.float64)
    batch = np.asarray(batch).astype(np.int64)
    logits = x @ np.asarray(W, np.float64)[:, 0] + float(np.asarray(b)[0])
    starts = np.searchsorted(batch, np.arange(B))
    counts = np.bincount(batch, minlength=B)
    # segment max (batch sorted -> reduceat over contiguous runs)
    valid = counts > 0
    seg_max = np.zeros(B)
    seg_max[valid] = np.maximum.reduceat(logits, starts[valid])[: valid.sum()]
    # reduceat quirk: rows with equal consecutive starts handled via `valid`
    e = np.exp(logits - seg_max[batch])
    seg_sum = np.zeros(B)
    seg_sum[valid] = np.add.reduceat(e, starts[valid])[: valid.sum()]
    w = e / (seg_sum[batch] + 1e-16)
    wx = w[:, None] * x
    out = np.zeros((B, x.shape[1]))
    out[valid] = np.add.reduceat(wx, starts[valid], axis=0)[: valid.sum()]
    return out


def kernel(x, batch, W, b, num_graphs):
    B = int(num_graphs)
    ref = _ref_numpy(x, batch, W, b, B)
    scale = max(1e-30, float(np.abs(ref).max()))
    best = None
    for mm in ("bf16", "f32"):
        in_maps, meta = prepare_shards(x, batch, W, b, B, mm_dtype=mm)
        nc = build_program(meta["T"], meta["G"], meta["S"], meta["n_groups"],
                           mm_dtype=mm)
        res = run_bass_kernel_spmd(nc, in_maps, core_ids=list(range(N_CORES)))
        out = np.concatenate(
            [res.results[c]["out"] for c in range(N_CORES)], axis=0
        ).astype(np.float32)
        rel = float(np.abs(np.asarray(out, np.float64) - ref).max() / scale)
        if best is None or rel < best[1]:
            best = (out, rel)
        if rel < 1.1e-2:
            return out
    return best[0]


# revision 31
# speedup vs baseline: 1.3932x; 1.1670x over previous
"""AttentionPool (segment softmax + weighted scatter-add) on 8 trn2 NeuronCores.

Strategy
--------
Segment-ALIGNED sharding: batch ids are sorted, and B = 1024 = 8 * 128, so
core c owns segments [128c, 128(c+1)) exactly.  Host computes the row range
of each core with searchsorted, so no cross-core collective is needed at all
-- each core produces a disjoint (128, 128) slice of the output.

Everything is bf16 on the wire and in the matmuls (the rel-err budget is
2e-2; measured ~6e-3): halves the HBM traffic (the roofline here, ~110 us
for 32.5 MB/core) and gives 1 cyc/row PE matmuls (fp32 is 4; float32r and
custom-DVE ops hang under this runtime; GpSimd scalar_tensor_tensor errors).

Per core, T=~981 row-tiles of 128 nodes, in groups of G=64 tiles (S=16
one-hot slots per group; the host guarantees each group's segment span
fits in S and provides slot ids):
  1. DMA the group's packed x (128, G*130 bf16: 128 x-cols + ones + pad),
     6 groups in flight (bufs_x=6) to keep the DMA engines saturated.
  2. logits: ONE fused DVE scalar_tensor_tensor per tile,
     scr = (x*1)*Wrep with accum_out -> l[p] = sum_d x[p,d]*W[d].
     (Fused beats any wide multiply+reduce 2-pass: it touches x once.
     On this runtime DVE wide ops run near 1 elem/lane/cycle and
     per-instruction issue overhead is ~40-60 ns.)
  3. e = exp(l + b): one ACT instruction per group (bf16 out).
  4. one-hot build: 2 WIDE DVE ops per group (s-major, all operands
     packed; broadcasts are middle-axis stride-0):
       oh0[p, s*G+t] = (iota_rep[p, s*G+t] == slot[p, t])
       oh[p, s*G+t]  = oh0[p, s*G+t] * e[p, t]
  5. PE: psum (S, 130) += oh_t^T @ [x | 1] per tile (lhsT is the s-major
     strided slice; Ldweights handles it at no measurable cost).
  6. per group: ACT-copy psum -> bf16 staging, then scatter-add into the
     (128, 130) fps psum with a small bf16 matmul against a one-hot from
     `fohs` (ALL groups' final one-hots are built in one wide DVE op
     hoisted out of the loop).
  7. final: out = v / (seg_sum + 1e-16); DMA out.

Measured (loop-delta over a x1000 hardware loop, device-resident inputs):
~211-292 us/invocation depending on device drift (DMA-only floor ~110 us),
vs 471 us for the f32 baseline this session started from.  Paired
in-process A/B runs are the only reliable way to compare variants; the
tunneled device drifts ~20% between processes.

The kernel() entry point takes FULL inputs and returns the FULL (1024, 128)
output; it validates the device result against a float64 numpy reference
on the host and falls back to the exact-f32 numeric config if the bf16
gate fails.
"""

import os
import sys

import numpy as np

for _p in ("/root/.axon_site", "/root/.axon_site/_ro/trn_rl_repo", "/root/.axon_site/_ro/pypackages"):
    if os.path.isdir(_p) and _p not in sys.path:
        sys.path.append(_p)

from contextlib import ExitStack

import ml_dtypes

import concourse.bacc as bacc
import concourse.tile as tile
from concourse import mybir
from concourse.bass_utils import run_bass_kernel_spmd

N_CORES = 8
D = 128
TPT = 130  # columns per tile in the packed x layout: 128 x + 1 ones + 1 pad

Alu = mybir.AluOpType
Act = mybir.ActivationFunctionType
F32 = mybir.dt.float32
BF16 = mybir.dt.bfloat16
NP_BF16 = ml_dtypes.bfloat16

_program_cache: dict = {}


def _b3(ap, S):
    """(P, T) AP -> (P, T, S) with stride-0 broadcast along s."""
    return ap.unsqueeze(2).broadcast_to([ap.shape[0], ap.shape[1], S])


def build_program(T, G, S, n_groups, mm_dtype="bf16", n_dma_per_group=2,
                  reps=1, bufs_x=6, reduce_mode="stt", oh_major="s",
                  pipelined=False, ladder=7, oh_engine="vector"):
    """Build the per-core bass program (same program for all 8 cores).

    reduce_mode: "tree" = log2 ladder of packed bf16 adds (DVE fast mode);
                 "tr"   = single 3D tensor_reduce (runs at 1x rate).
    oh_major:    "s" = s-major one-hot (strided matmul lhsT);
                 "t" = t-major (contiguous lhsT, stride-0 inner broadcasts).
    pipelined:   emit group g's e-scale/matmuls during group g+1 so the
                 in-order DVE/PE streams never stall on the ACT exp/copy.
    """
    key = (T, G, S, n_groups, mm_dtype, n_dma_per_group, reps, bufs_x,
           reduce_mode, oh_major, pipelined, ladder, oh_engine)
    if key in _program_cache:
        return _program_cache[key]

    assert n_groups == (T + G - 1) // G
    nc = bacc.Bacc("TRN2", target_bir_lowering=False)

    bf16 = mm_dtype == "bf16"
    XDT = BF16 if bf16 else F32

    x_in = nc.declare_dram_parameter("xs", [128, T * TPT], XDT, isOutput=False)
    slots_in = nc.declare_dram_parameter("slots", [128, T], XDT, isOutput=False)
    fslots_in = nc.declare_dram_parameter("fslots", [S, n_groups], F32, isOutput=False)
    wrep_in = nc.declare_dram_parameter("wrep", [128, TPT], XDT, isOutput=False)
    brep_in = nc.declare_dram_parameter("brep", [128, 1], F32, isOutput=False)
    iota_rep_in = nc.declare_dram_parameter("iota_rep", [128, S * G], XDT, isOutput=False)
    iota_rep_t_in = nc.declare_dram_parameter("iota_rep_t", [128, G * S], XDT, isOutput=False)
    iota_m_in = nc.declare_dram_parameter("iota_m", [S, 128], XDT, isOutput=False)
    y_out = nc.declare_dram_parameter("out", [128, 128], F32, isOutput=True)

    with tile.TileContext(nc) as tc:
        with ExitStack() as ctx:
            cpool = ctx.enter_context(tc.tile_pool(name="consts", bufs=1))
            xpool = ctx.enter_context(tc.tile_pool(name="x", bufs=bufs_x))
            spool = ctx.enter_context(tc.tile_pool(name="scr", bufs=4))
            tpool = ctx.enter_context(tc.tile_pool(name="tree", bufs=1))
            lpool = ctx.enter_context(tc.tile_pool(name="l", bufs=2))
            epool = ctx.enter_context(tc.tile_pool(name="e", bufs=2))
            oh0pool = ctx.enter_context(tc.tile_pool(name="oh0", bufs=2))
            ohpool = ctx.enter_context(tc.tile_pool(name="oh", bufs=2))
            pspool = ctx.enter_context(tc.tile_pool(name="ps", bufs=4, space="PSUM"))
            stpool = ctx.enter_context(tc.tile_pool(name="stage", bufs=2))
            fpool = ctx.enter_context(tc.tile_pool(name="fin", bufs=1, space="PSUM"))
            opool = ctx.enter_context(tc.tile_pool(name="outp", bufs=1))

            wrep = cpool.tile([128, TPT], XDT)
            nc.sync.dma_start(wrep[:], wrep_in[:])
            brep = cpool.tile([128, 1], F32)
            nc.sync.dma_start(brep[:], brep_in[:])
            iota_rep = cpool.tile([128, S * G], XDT)
            nc.sync.dma_start(iota_rep[:], iota_rep_in[:])
            iota_rep_t = cpool.tile([128, G * S], XDT)
            nc.sync.dma_start(iota_rep_t[:], iota_rep_t_in[:])
            iota_m = cpool.tile([S, 128], XDT)
            nc.sync.dma_start(iota_m[:], iota_m_in[:])
            slots = cpool.tile([128, T], XDT)
            nc.sync.dma_start(slots[:], slots_in[:])
            fslots = cpool.tile([S, n_groups], F32)
            nc.sync.dma_start(fslots[:], fslots_in[:])

            # all groups' final-scatter one-hots in ONE wide op, hoisted out
            # of the reps loop entirely (depends only on consts)
            fohs = cpool.tile([S, n_groups * 128], XDT)
            nc.vector.scalar_tensor_tensor(
                fohs[:].rearrange("p (g m) -> p g m", m=128),
                iota_m[:].unsqueeze(1).broadcast_to([S, n_groups, 128]),
                1.0,
                _b3(fslots[:], 128),
                Alu.mult,
                Alu.is_equal,
            )

            def _front_tail(g, Gg, xc, l_t):
                if ladder < 3:
                    return dict(g=g, Gg=Gg, xc=xc, oh0=None, e_t=None)
                e_t = epool.tile([128, Gg], XDT, tag="e")
                nc.scalar.activation(e_t[:], l_t[:], Act.Exp, bias=brep[:], scale=1.0)
                if ladder < 4:
                    return dict(g=g, Gg=Gg, xc=xc, oh0=None, e_t=e_t)
                # unscaled one-hot (depends only on consts)
                oh0 = oh0pool.tile([128, S * G], XDT, tag="oh0")
                ohe = nc.gpsimd if oh_engine == "gpsimd" else nc.vector
                if oh_major == "s":
                    # oh0[p, s*G + t] = (iota_rep[p, s*G+t] == slot[p, t])
                    ohe.scalar_tensor_tensor(
                        oh0[:].rearrange("p (s t) -> p s t", t=G)[:, :, 0:Gg],
                        iota_rep[:].rearrange("p (s t) -> p s t", t=G)[:, :, 0:Gg],
                        1.0,
                        slots[:, g * G : g * G + Gg].unsqueeze(1).broadcast_to([128, S, Gg]),
                        Alu.mult,
                        Alu.is_equal,
                    )
                else:
                    # oh0[p, t*S + s] = (iota_rep_t[p, t*S+s] == slot[p, t])
                    ohe.scalar_tensor_tensor(
                        oh0[:, 0 : Gg * S].rearrange("p (t s) -> p t s", s=S),
                        iota_rep_t[:, 0 : Gg * S].rearrange("p (t s) -> p t s", s=S),
                        1.0,
                        _b3(slots[:, g * G : g * G + Gg], S),
                        Alu.mult,
                        Alu.is_equal,
                    )
                return dict(g=g, Gg=Gg, xc=xc, oh0=oh0, e_t=e_t)

            def emit_front_half(g):
                """DMA + logits (mult, reduce) + exp + unscaled one-hot for
                group g.  Everything here depends only on xc/consts, so the
                in-order DVE stream never waits on another engine."""
                Gg = min(G, T - g * G)
                xc = xpool.tile([128, G * TPT], XDT, tag="xc")
                cols = Gg * TPT
                step = (cols + n_dma_per_group - 1) // n_dma_per_group
                for k in range(0, cols, step):
                    w = min(step, cols - k)
                    nc.sync.dma_start(
                        xc[:, k : k + w],
                        x_in[:, g * G * TPT + k : g * G * TPT + k + w],
                    )
                if ladder < 2:
                    return dict(g=g, Gg=Gg, xc=xc, oh0=None, e_t=None)
                if reduce_mode == "stt":
                    # fused multiply+accumulate per tile: ONE pass over x
                    # (the 2-pass wide variants touch the data twice)
                    l_t = lpool.tile([128, Gg], F32, tag="l")
                    for t in range(Gg):
                        scr = spool.tile([128, 128], XDT, tag="scr")
                        nc.vector.scalar_tensor_tensor(
                            scr[:],
                            xc[:, t * TPT : t * TPT + 128],
                            1.0,
                            wrep[:, 0:128],
                            Alu.mult,
                            Alu.mult,
                            accum_out=l_t[:, t : t + 1],
                        )
                    return _front_tail(g, Gg, xc, l_t)
                # logits pass 1: scr = x * Wrep (all operands packed bf16;
                # W pattern has zeros at the ones/pad columns)
                scr_w = spool.tile([128, G * TPT], XDT, tag="scrw")
                scr3 = scr_w[:, 0 : Gg * TPT].rearrange("p (t c) -> p t c", c=TPT)
                nc.vector.scalar_tensor_tensor(
                    scr3,
                    xc[:, 0 : Gg * TPT].rearrange("p (t c) -> p t c", c=TPT),
                    1.0,
                    wrep[:].unsqueeze(1).broadcast_to([128, Gg, TPT]),
                    Alu.mult,
                    Alu.mult,
                )
                # logits pass 2: per-tile row sums of the first 128 columns
                l_t = lpool.tile([128, Gg], F32, tag="l")
                if reduce_mode == "tr":
                    nc.vector.tensor_reduce(
                        l_t[:], scr3, mybir.AxisListType.X, Alu.add
                    )
                else:
                    # log2 ladder of packed bf16 adds (DVE fast mode);
                    # level widths 64,32,16,8,4,2 then a final fp32 add.
                    tree = tpool.tile([128, G * 126], XDT, tag="tree")
                    off = 0
                    src3 = scr3
                    w = 64
                    while w >= 2:
                        dst = tree[:, off * G : off * G + Gg * w]
                        dst3 = dst.rearrange("p (t c) -> p t c", c=w)
                        nc.vector.scalar_tensor_tensor(
                            dst3,
                            src3[:, :, 0:w],
                            1.0,
                            src3[:, :, w : 2 * w],
                            Alu.mult,
                            Alu.add,
                        )
                        src3 = dst3
                        off += w
                        w //= 2
                    nc.vector.scalar_tensor_tensor(
                        l_t[:].unsqueeze(2),
                        src3[:, :, 0:1],
                        1.0,
                        src3[:, :, 1:2],
                        Alu.mult,
                        Alu.add,
                    )
                return _front_tail(g, Gg, xc, l_t)

            def emit_back_half(st):
                """e-scaling + scatter matmuls + fold-in for group st['g'].
                Emitted one group late so exp/copy results are ready."""
                g, Gg, xc, oh0, e_t = st["g"], st["Gg"], st["xc"], st["oh0"], st["e_t"]
                if ladder < 5:
                    return None
                oh = ohpool.tile([128, S * G], XDT, tag="oh")
                ohe = nc.gpsimd if oh_engine == "gpsimd" else nc.vector
                if oh_major == "s":
                    ohe.scalar_tensor_tensor(
                        oh[:].rearrange("p (s t) -> p s t", t=G)[:, :, 0:Gg],
                        oh0[:].rearrange("p (s t) -> p s t", t=G)[:, :, 0:Gg],
                        1.0,
                        e_t[:].unsqueeze(1).broadcast_to([128, S, Gg]),
                        Alu.mult,
                        Alu.mult,
                    )
                else:
                    ohe.scalar_tensor_tensor(
                        oh[:, 0 : Gg * S].rearrange("p (t s) -> p t s", s=S),
                        oh0[:, 0 : Gg * S].rearrange("p (t s) -> p t s", s=S),
                        1.0,
                        _b3(e_t[:], S),
                        Alu.mult,
                        Alu.mult,
                    )
                if ladder < 6:
                    return None
                ps = pspool.tile([S, TPT], F32, tag="ps")
                for t in range(Gg):
                    if oh_major == "s":
                        lhsT = oh[:, t : t + (S - 1) * G + 1 : G]
                    else:
                        lhsT = oh[:, t * S : (t + 1) * S]
                    nc.tensor.matmul(
                        ps[:],
                        lhsT=lhsT,
                        rhs=xc[:, t * TPT : t * TPT + TPT],
                        start=(t == 0),
                        stop=(t == Gg - 1),
                    )
                staging = stpool.tile([S, TPT], XDT, tag="stage")
                nc.scalar.copy(staging[:], ps[:])
                return dict(g=g, staging=staging)

            def emit_fold(fps, fin_st):
                if fin_st is None or ladder < 7:
                    return
                g, staging = fin_st["g"], fin_st["staging"]
                nc.tensor.matmul(
                    fps[:],
                    lhsT=fohs[:, g * 128 : (g + 1) * 128],
                    rhs=staging[:],
                    start=(g == 0),
                    stop=(g == n_groups - 1),
                )

            def emit_body():
                fps = fpool.tile([128, TPT], F32, tag="fps")
                if pipelined:
                    pending = None
                    pending_fold = None
                    for g in range(n_groups):
                        st = emit_front_half(g)
                        if pending is not None:
                            fin_st = emit_back_half(pending)
                            if pending_fold is not None:
                                emit_fold(fps, pending_fold)
                            pending_fold = fin_st
                        pending = st
                    fin_st = emit_back_half(pending)
                    if pending_fold is not None:
                        emit_fold(fps, pending_fold)
                    emit_fold(fps, fin_st)
                else:
                    for g in range(n_groups):
                        st = emit_front_half(g)
                        fin_st = emit_back_half(st)
                        emit_fold(fps, fin_st)
                if ladder < 7:
                    out_sb = opool.tile([128, 128], F32, tag="ot")
                    nc.vector.memset(out_sb[:], 0.0)
                    nc.sync.dma_start(y_out[:], out_sb[:])
                    return
                s_plus = opool.tile([128, 1], F32, tag="sp")
                nc.vector.tensor_scalar_add(s_plus[:], fps[:, 128:129], 1e-16)
                recip = opool.tile([128, 1], F32, tag="rc")
                nc.vector.reciprocal(recip[:], s_plus[:])
                out_sb = opool.tile([128, 128], F32, tag="ot")
                nc.vector.tensor_scalar(
                    out_sb[:], fps[:, 0:128], recip[:], None, Alu.mult
                )
                nc.sync.dma_start(y_out[:], out_sb[:])

            if reps == 1:
                emit_body()
            else:
                with tc.For_i(0, reps, 1):
                    emit_body()

    nc.finalize()
    _program_cache[key] = nc
    return nc


def prepare_shards(x, batch, W, b, B, S=16, G=64, mm_dtype="bf16"):
    """Host-side packing. Returns (in_maps, meta)."""
    x = np.asarray(x, dtype=np.float32)
    batch = np.asarray(batch).astype(np.int64)
    W = np.asarray(W, dtype=np.float32)
    b = np.asarray(b, dtype=np.float32)
    np_xdt = NP_BF16 if mm_dtype == "bf16" else np.float32
    N = x.shape[0]
    segs_per_core = B // N_CORES
    bounds = np.searchsorted(batch, np.arange(0, B + 1, segs_per_core))
    T = int(max(-(-(int(bounds[c + 1] - bounds[c])) // 128) for c in range(N_CORES)))

    # pick G such that every group's segment span fits in S slots
    loc_all = batch - (batch // segs_per_core) * segs_per_core
    while G > 1:
        ok = True
        for c in range(N_CORES):
            r0, r1 = int(bounds[c]), int(bounds[c + 1])
            n = r1 - r0
            if n == 0:
                continue
            loc = loc_all[r0:r1]
            g_idx = np.arange(n) // (G * 128)
            gstart = np.minimum(np.arange(g_idx[-1] + 1) * G * 128, n - 1)
            gb = loc[gstart]
            span = loc - gb[g_idx]
            if span.min() < 0 or span.max() >= S:
                ok = False
                break
        if ok:
            break
        G //= 2
    n_groups = (T + G - 1) // G

    wpat = np.zeros(TPT, np.float32)
    wpat[:128] = W[:, 0]
    wrep = np.tile(wpat[None, :], (128, 1)).astype(np_xdt)
    brep = np.full((128, 1), float(b[0]), np.float32)
    # s-major iota: value s at position s*G + t
    iota_rep = np.tile(
        np.repeat(np.arange(S, dtype=np.float32), G)[None, :], (128, 1)
    ).astype(np_xdt)
    # t-major iota: value s at position t*S + s
    iota_rep_t = np.tile(
        np.arange(S, dtype=np.float32)[None, :], (128, G)
    ).astype(np_xdt)
    iota_m = np.tile(np.arange(128, dtype=np.float32)[None, :], (S, 1)).astype(np_xdt)

    in_maps = []
    for c in range(N_CORES):
        r0, r1 = int(bounds[c]), int(bounds[c + 1])
        n = r1 - r0
        xp = np.zeros((T * 128, TPT), np_xdt)
        xp[:n, :128] = x[r0:r1].astype(np_xdt)
        xp[:n, 128] = 1.0
        x_shard = np.ascontiguousarray(
            xp.reshape(T, 128, TPT).transpose(1, 0, 2).reshape(128, T * TPT)
        )

        slots_full = np.full(T * 128, -1.0, np.float32)
        fslots = np.full((S, n_groups), -1.0, np.float32)
        if n > 0:
            loc = loc_all[r0:r1]
            g_idx = np.arange(n) // (G * 128)
            ng_real = int(g_idx[-1]) + 1
            gstart = np.minimum(np.arange(ng_real) * G * 128, n - 1)
            gb = loc[gstart]
            slot = loc - gb[g_idx]
            assert slot.min() >= 0 and slot.max() < S
            slots_full[:n] = slot.astype(np.float32)  # ints <= S fit bf16 exactly
            for g in range(ng_real):
                segs = gb[g] + np.arange(S)
                valid = segs < segs_per_core
                fslots[valid, g] = segs[valid].astype(np.float32)
        slots_T = np.ascontiguousarray(slots_full.reshape(T, 128).T).astype(np_xdt)

        in_maps.append(
            {
                "xs": x_shard,
                "slots": slots_T,
                "fslots": fslots,
                "wrep": wrep,
                "brep": brep,
                "iota_rep": iota_rep,
                "iota_rep_t": iota_rep_t,
                "iota_m": iota_m,
            }
        )
    meta = dict(T=T, G=G, S=S, n_groups=n_groups, segs_per_core=segs_per_core)
    return in_maps, meta


def _ref_numpy(x, batch, W, b, B):
    """Float64 host reference (same math as the jax oracle) used only as a
    validation gate for the on-device numeric mode."""
    x = np.asarray(x, np.float64)
    batch = np.asarray(batch).astype(np.int64)
    logits = x @ np.asarray(W, np.float64)[:, 0] + float(np.asarray(b)[0])
    starts = np.searchsorted(batch, np.arange(B))
    counts = np.bincount(batch, minlength=B)
    # segment max (batch sorted -> reduceat over contiguous runs)
    valid = counts > 0
    seg_max = np.zeros(B)
    seg_max[valid] = np.maximum.reduceat(logits, starts[valid])[: valid.sum()]
    # reduceat quirk: rows with equal consecutive starts handled via `valid`
    e = np.exp(logits - seg_max[batch])
    seg_sum = np.zeros(B)
    seg_sum[valid] = np.add.reduceat(e, starts[valid])[: valid.sum()]
    w = e / (seg_sum[batch] + 1e-16)
    wx = w[:, None] * x
    out = np.zeros((B, x.shape[1]))
    out[valid] = np.add.reduceat(wx, starts[valid], axis=0)[: valid.sum()]
    return out


def kernel(x, batch, W, b, num_graphs):
    B = int(num_graphs)
    ref = _ref_numpy(x, batch, W, b, B)
    scale = max(1e-30, float(np.abs(ref).max()))
    best = None
    for mm in ("bf16", "f32"):
        in_maps, meta = prepare_shards(x, batch, W, b, B, mm_dtype=mm)
        nc = build_program(meta["T"], meta["G"], meta["S"], meta["n_groups"],
                           mm_dtype=mm)
        res = run_bass_kernel_spmd(nc, in_maps, core_ids=list(range(N_CORES)))
        out = np.concatenate(
            [res.results[c]["out"] for c in range(N_CORES)], axis=0
        ).astype(np.float32)
        rel = float(np.abs(np.asarray(out, np.float64) - ref).max() / scale)
        if best is None or rel < best[1]:
            best = (out, rel)
        if rel < 1.1e-2:
            return out
    return best[0]
